# revision 1
# baseline (speedup 1.0000x reference)
"""HMM-style alpha-recursion kernel for nn_Net_11553462026249.

Strategy (per sharding_hint): data-parallel over batch N across the 8
NeuronCores — the alpha recursion is independent per sequence. Each core
holds N/8 = 8 sequences, replicates the small weights (emb, vocab_w,
emb_cluster_w, start, trans_w), computes its partial mean of the final
logsumexp, and the host averages the 8 equal-sized partials.

Shapes are hardcoded per the problem spec:
  w:(64,128) int, emb:(32000,100), vocab_w:(32000,256),
  emb_cluster_w:(256,256), start_w:(256,1), start_b:(256,),
  trans_w:(65536,100). Output: scalar float32.
"""

import functools

import numpy as np
import jax
import jax.numpy as jnp
from jax import lax

N, T, K, V, E = 64, 128, 256, 32000, 100
NC = 8


@functools.lru_cache(maxsize=1)
def _get_forward_shard():
    return jax.pmap(
        _forward_shard_impl,
        in_axes=(0, None, None, None, None, None, None),
        devices=jax.devices()[:NC],
    )


def _forward_shard_impl(w_l, emb, vocab_w, emb_cluster_w, start_w, start_b, trans_w):
    n, t = w_l.shape
    k = emb_cluster_w.shape[0]
    x = emb[w_l]                                            # (n,T,E)
    pre_alpha = jnp.broadcast_to(
        jax.nn.log_softmax(start_w[:, 0] + start_b), (n, k)
    )
    log_em_t = jax.nn.log_softmax(emb_cluster_w @ vocab_w.T, axis=-1).T  # (V,K)

    def step(alpha, inputs):
        x_prev, w_t = inputs                                # (n,E), (n,)
        tran = jax.nn.log_softmax(
            (x_prev @ trans_w.T).reshape(n, k, k), axis=-1
        )                                                   # (n,K,K)
        a = jax.nn.logsumexp(alpha[:, :, None] + tran, axis=1)
        a = a + log_em_t[w_t]
        return a, None

    xs = (jnp.swapaxes(x[:, :-1, :], 0, 1), w_l[:, 1:].T)
    alpha, _ = lax.scan(step, pre_alpha, xs)
    return jnp.mean(jax.nn.logsumexp(alpha, axis=1))        # per-shard mean


def _np_logsumexp(a, axis):
    m = np.max(a, axis=axis, keepdims=True)
    return (m + np.log(np.sum(np.exp(a - m), axis=axis, keepdims=True))).squeeze(axis)


def _np_forward(w, emb, vocab_w, emb_cluster_w, start_w, start_b, trans_w):
    n, t = w.shape
    k = emb_cluster_w.shape[0]
    x = emb[w]
    s = start_w[:, 0] + start_b
    alpha = np.broadcast_to(s - _np_logsumexp(s, 0), (n, k)).copy()
    logits = emb_cluster_w @ vocab_w.T                      # (K,V)
    log_em_t = (logits - _np_logsumexp(logits, 1)[:, None]).T  # (V,K)
    for step in range(1, t):
        L = (x[:, step - 1, :] @ trans_w.T).reshape(n, k, k)
        L -= _np_logsumexp(L, 2)[:, :, None]
        alpha = _np_logsumexp(alpha[:, :, None] + L, axis=1) + log_em_t[w[:, step]]
    return np.float32(-np.mean(_np_logsumexp(alpha, 1)))


def kernel(w, emb, vocab_w, emb_cluster_w, start_w, start_b, trans_w):
    w = np.asarray(w).astype(np.int32)
    emb = np.asarray(emb, dtype=np.float32)
    vocab_w = np.asarray(vocab_w, dtype=np.float32)
    emb_cluster_w = np.asarray(emb_cluster_w, dtype=np.float32)
    start_w = np.asarray(start_w, dtype=np.float32)
    start_b = np.asarray(start_b, dtype=np.float32)
    trans_w = np.asarray(trans_w, dtype=np.float32)

    try:
        parts = _get_forward_shard()(
            w.reshape(NC, N // NC, T), emb, vocab_w, emb_cluster_w,
            start_w, start_b, trans_w,
        )
        # Equal shard sizes -> mean of per-shard means == global mean.
        return np.float32(-np.mean(np.asarray(parts)))
    except Exception:
        return _np_forward(
            w, emb, vocab_w, emb_cluster_w, start_w, start_b, trans_w
        )

